# revision 66
# baseline (speedup 1.0000x reference)
"""Trainium2 Bass kernel for GCE-TAGNN session recommendation model.

Strategy: batch-sharded, collective-free.
  - Each core owns 8 sessions and scores them against ALL 10240 (padded)
    candidates: no all-gather, no barrier, no launch-skew sensitivity.
  - Global GNN: hg is only consumed as hg[session_items], so each core
    aggregates ONLY the edges targeting its own sessions' items (host-routed
    per position slot) and applies gW/relu locally -> sess_glob directly.
  - Session adjacency matmuls are transpose-free: Y^T computed directly via
    matmul with h0T as weights, then block-diagonal (2 sessions) adj matmul.
  - MHA batched across all 8 local sessions using a head-replicated
    block-diagonal matmul; softmax pipeline runs on [128, 400] tiles.
  - Target attention: with d = cand @ w3_W ([N,384]),
      scores[b,n] = (sum_l E*g)/(sum_l E) + last[b]*d[n,128:256]
                    + s_global[b]*d[n,256:384]
    ts = final·(w_target_W cand[n]), E = exp(ts) (|ts| tiny, no max needed),
    g = final·d[n,:128].  trT/c0 transforms precomputed on host (fp16).
    last/sglo terms = cand[n]·u_b with u = w3_2 last + w3_3 sglo: emitted as
    20 wide [8,512] matmuls in [b, n] layout; the HOST adds them to the
    device's [n, b] softmax term (free transpose-avoidance).
    Per-b softmax denominator corrected by subtracting (L - len[b]).
    The first 24 candidate chunks use a finite-difference trick to avoid the
    elementwise E*g product entirely (balancing Vector vs Scalar):
      sum_l E*g = (sum_l exp(ts+eps*g) - sum_l exp(ts-eps*g)) / (2*eps)
    with host-precombined weight tables trT +- eps*c0 and fp32 accumulation.
    Engine split in phase D: exp on Scalar, E*g on Vector, 50->25 folds on
    GpSimd, 25-col segment reductions on Vector, softmax tails half-hidden
    under the main loop.
"""

import sys

sys.path.insert(0, "/opt/trn_rl_repo")

import math

import numpy as np

import concourse.mybir as mybir
import concourse.tile as tile
from concourse import bacc
from concourse.bass_utils import run_bass_kernel_spmd

F32 = mybir.dt.float32
F32R = mybir.dt.float32r
F16 = mybir.dt.float16
I32 = mybir.dt.int32
AX = mybir.AxisListType
ALU = mybir.AluOpType
ACT = mybir.ActivationFunctionType

NC = 8          # cores
B = 64          # batch
L = 50          # session length
H = 128         # hidden
NH = 8          # heads
NIT = 10000     # item vocab
NPAD = 10240    # padded vocab
NCH = NPAD // H  # 80 candidate chunks of 128
BLOC = B // NC  # sessions per core
RL = BLOC * L   # 400 position slots per core
WINA = 16       # agg position window
NWINA = RL // WINA  # 25 windows per core

# ---- packed-constant column offsets ----
_OF_F = {}
_o = 0
for _n, _w in [("blinrow", H), ("bloutrow", H),
               ("bih", 3), ("bhh", 3), ("prjb", 3), ("oprjb", 1), ("gb", 1)]:
    _OF_F[_n] = _o
    _o += _w
PF = _o

_OF_M = {}
_o = 0
for _n, _w in [("attm", RL), ("colm", RL), ("lastsel", RL), ("npadl", 32)]:
    _OF_M[_n] = _o
    _o += _w
PM = _o

_OF_R = {}
_o = 0
for _n, _w in [("linT", H), ("loutT", H), ("whh", 3 * H), ("prjT", 3 * H),
               ("oprjT", H), ("wih", 6 * H)]:
    _OF_R[_n] = _o
    _o += _w
PR = _o

_OF_H = {}
_o = 0
for _n, _w in [("w32T", H), ("w33T", H), ("gWT", H), ("bd128", H)]:
    _OF_H[_n] = _o
    _o += _w
PH = _o

NGPM = 8         # leading D groups using the +/-eps finite-difference path
NPM = NGPM * 4   # chunks on that path
NDIR = NPAD - NPM * H   # direct-path candidate columns
NPH = NDIR // 2  # direct-path half-width for priority-ordered uploads
EPS = 2.4        # finite-difference step: E*g = (E+ - E-)/(2*EPS)

_NC_CACHE = {}


def build_nc(T):
    """Build the per-core program. T = edge tiles per position window."""
    NT = NWINA * T  # edge tiles per core
    nc = bacc.Bacc(None, target_bir_lowering=False)

    def inp(name, shape, dtype=F32):
        return nc.dram_tensor(name, shape, dtype, kind="ExternalInput")

    h0x_d = inp("h0x", [H, RL], F32R)   # emb[items]^T, host-gathered
    pox_d = inp("pox", [H, RL])         # pos_emb[rev]^T, host-gathered
    packf_d = inp("packf", [H, PF])
    packm_d = inp("packm", [H, PM])
    packr_d = inp("packr", [H, PR], F32R)
    packh_d = inp("packh", [H, PH], F16)
    trTa_d = inp("trTa", [H, NPH], F16)
    trTb_d = inp("trTb", [H, NPH], F16)
    c0a_d = inp("c0a", [H, NPH], F16)
    c0b_d = inp("c0b", [H, NPH], F16)
    candt_d = inp("candt", [H, NPAD], F16)
    trTp_d = inp("trTp", [H, NPM * H], F16)
    trTm_d = inp("trTm", [H, NPM * H], F16)
    adjbd_d = inp("adjbd", [BLOC // 2, 2 * L, 2 * L], F32R)
    eemb = inp("eemb", [H, NT, H], F16)
    swt = inp("swt", [H, NT, WINA], F16)

    s1_out = nc.dram_tensor("scores1", [H, NCH * BLOC], F32,
                            kind="ExternalOutput")
    t23_out = nc.dram_tensor("t23", [NH, NPAD], F32, kind="ExternalOutput")

    with tile.TileContext(nc) as tc:
        with (
            tc.tile_pool(name="cst", bufs=1) as cst,
            tc.tile_pool(name="wk", bufs=3) as wk,
            tc.tile_pool(name="pp", bufs=8, space="PSUM") as pp,
        ):
            def psum(shape, tag="ps", dtype=F32):
                nbuf = {"ps": 2, "ts": 3, "gg": 3}[tag]
                return pp.tile(shape, dtype, tag=tag, name=tag, bufs=nbuf)

            # ---------- constant loads (packed); big packh goes LAST so
            # phase A/B inputs aren't queued behind it ----------
            h0T = cst.tile([H, RL], F32R, name="h0T")
            nc.sync.dma_start(h0T[:], h0x_d[:])
            adjbd = cst.tile([2 * L, BLOC // 2, 2 * L], F32R, name="adjbd")
            nc.sync.dma_start(adjbd[:], adjbd_d.rearrange("j p k -> p j k"))
            packf = cst.tile([H, PF], F32, name="packf")
            nc.sync.dma_start(packf[:], packf_d[:])
            packr = cst.tile([H, PR], F32R, name="packr")
            nc.sync.dma_start(packr[:], packr_d[:])
            emA = wk.tile([H, NT, H], F16, tag="epTP", bufs=2)
            nc.sync.dma_start(emA[:], eemb[:])
            swA = wk.tile([H, NT, WINA], F16, tag="epTP", bufs=2)
            nc.sync.dma_start(swA[:], swt[:])
            poT = cst.tile([H, RL], F32, name="poT")
            nc.sync.dma_start(poT[:], pox_d[:])
            packm = cst.tile([H, PM], F32, name="packm")
            nc.sync.dma_start(packm[:], packm_d[:])
            packh = cst.tile([H, PH], F16, name="packh")
            nc.sync.dma_start(packh[:], packh_d[:])
            # candidate-side transforms, priority-ordered: first halves of
            # trT/c0 land first so phase D can start before the rest arrive
            trTh = [cst.tile([H, NPH], F16, name=f"trT{i}") for i in range(2)]
            c0h = [cst.tile([H, NPH], F16, name=f"c0{i}") for i in range(2)]
            trTp = cst.tile([H, NPM * H], F16, name="trTp")
            nc.sync.dma_start(trTp[:], trTp_d[:])
            trTm = cst.tile([H, NPM * H], F16, name="trTm")
            nc.sync.dma_start(trTm[:], trTm_d[:])
            nc.sync.dma_start(trTh[0][:], trTa_d[:])
            nc.sync.dma_start(c0h[0][:], c0a_d[:])
            nc.sync.dma_start(trTh[1][:], trTb_d[:])
            nc.sync.dma_start(c0h[1][:], c0b_d[:])


            def fview(name, w=None):
                if name in _OF_M:
                    o = _OF_M[name]
                    return packm[:, o:o + (w if w is not None else 1)]
                o = _OF_F[name]
                return packf[:, o:o + (w if w is not None else 1)]

            def rview(name, off=0, w=H):
                return packr[:, _OF_R[name] + off:_OF_R[name] + off + w]

            def hview(name, off=0, w=H):
                return packh[:, _OF_H[name] + off:_OF_H[name] + off + w]

            # =======================================================
            # Phase B: session path (8 local sessions)
            # inp = adj @ (h W^T + b) via transpose-free block-diag matmuls
            # =======================================================
            iinT = cst.tile([H, RL], F32R, name="iinT")
            ioutT = cst.tile([H, RL], F32R, name="ioutT")
            for blk in range(4):
                sl = slice(blk * 2 * L, (blk + 1) * 2 * L)
                for wname, brow, dst in (("linT", "blinrow", iinT),
                                         ("loutT", "bloutrow", ioutT)):
                    ps_yt = psum([2 * L, H], tag="ps")
                    nc.tensor.matmul(ps_yt[:], h0T[:, sl], rview(wname))
                    yt = wk.tile([2 * L, H], F32R, tag="yt")
                    nc.vector.tensor_add(yt[:], ps_yt[:],
                                         packf[0:2 * L,
                                               _OF_F[brow]:_OF_F[brow] + H])
                    ps_ii = psum([H, 2 * L], tag="gg")
                    nc.tensor.matmul(ps_ii[:], yt[:], adjbd[:, blk, :])
                    nc.vector.tensor_copy(dst[:, sl], ps_ii[:])

            # =======================================================
            # Phase A: GNN aggregation for this core's session positions.
            # =======================================================
            agg_ps = psum([H, RL], tag="ts")
            for w in range(NWINA):
                for t in range(T):
                    j = w * T + t
                    nc.tensor.matmul(
                        agg_ps[:, w * WINA:(w + 1) * WINA],
                        emA[:, j, :], swA[:, j, :],
                        start=(t == 0), stop=(t == T - 1))
            aggA = cst.tile([H, RL], F16, name="aggA")
            nc.vector.tensor_copy(aggA[:], agg_ps[:])
            # sess_glob^T = relu(gW @ agg + gb) in position order
            sgA = cst.tile([H, RL], F32, name="sgA")
            ps_sga = psum([H, RL], tag="gg")
            nc.tensor.matmul(ps_sga[:], hview("gWT"), aggA[:])
            nc.scalar.activation(sgA[:], ps_sga[:], ACT.Relu,
                                 bias=fview("gb"))

            # GRU cell (feature-major)
            combR = cst.tile([H, 2], F32, name="combR")
            nc.vector.tensor_add(combR[:, 0:1], fview("bih"), fview("bhh"))
            nc.vector.tensor_add(combR[:, 1:2],
                                 packf[:, _OF_F["bih"] + 1:_OF_F["bih"] + 2],
                                 packf[:, _OF_F["bhh"] + 1:_OF_F["bhh"] + 2])
            gates = []
            for g in range(2):  # r, z
                ps_gate = psum([H, RL], tag="ts")
                nc.tensor.matmul(ps_gate[:], rview("wih", g * H),
                                 iinT[:], start=True, stop=False)
                nc.tensor.matmul(ps_gate[:], rview("wih", 3 * H + g * H),
                                 ioutT[:], start=False, stop=False)
                nc.tensor.matmul(ps_gate[:], rview("whh", g * H),
                                 h0T[:], start=False, stop=True)
                gt = cst.tile([H, RL], F32, name=f"gate{g}")
                nc.scalar.activation(gt[:], ps_gate[:], ACT.Sigmoid,
                                     bias=combR[:, g:g + 1])
                gates.append(gt)
            rT, zT = gates
            ps_in = psum([H, RL], tag="ts")
            nc.tensor.matmul(ps_in[:], rview("wih", 2 * H), iinT[:],
                             start=True, stop=False)
            nc.tensor.matmul(ps_in[:], rview("wih", 5 * H), ioutT[:],
                             start=False, stop=True)
            ps_hn = psum([H, RL], tag="gg")
            nc.tensor.matmul(ps_hn[:], rview("whh", 2 * H), h0T[:])
            rhn = cst.tile([H, RL], F32, name="rhn")
            nc.vector.scalar_tensor_tensor(
                out=rhn[:], in0=ps_hn[:],
                scalar=packf[:, _OF_F["bhh"] + 2:_OF_F["bhh"] + 3],
                in1=rT[:], op0=ALU.add, op1=ALU.mult)
            tmp_n = cst.tile([H, RL], F32, name="tmp_n")
            nc.vector.tensor_add(tmp_n[:], ps_in[:], rhn[:])
            nT = cst.tile([H, RL], F32, name="nT")
            nc.scalar.activation(nT[:], tmp_n[:], ACT.Tanh,
                                 bias=packf[:, _OF_F["bih"] + 2:
                                            _OF_F["bih"] + 3])
            diff = cst.tile([H, RL], F32, name="diff")
            nc.vector.tensor_sub(diff[:], h0T[:], nT[:])
            zd = cst.tile([H, RL], F32, name="zd")
            nc.vector.tensor_mul(zd[:], zT[:], diff[:])
            h1T = cst.tile([H, RL], F32, name="h1T")
            nc.vector.tensor_add(h1T[:], nT[:], zd[:])

            # rich = sess_glob + h1; final = (rich + pos_emb[rev]) * colmask
            richT = cst.tile([H, RL], F32, name="richT")
            nc.vector.tensor_add(richT[:], h1T[:], sgA[:])
            finT = cst.tile([H, RL], F32R, name="finT")
            nc.vector.tensor_add(finT[:], richT[:], poT[:])
            nc.vector.tensor_mul(finT[:], finT[:], fview("colm", RL))
            fin16 = cst.tile([H, RL], F16, name="fin16")
            nc.vector.tensor_copy(fin16[:], finT[:])

            # last[b] = final[b, len_b - 1]  (one-hot selection + reduce)
            lsel = cst.tile([H, RL], F32, name="lsel")
            nc.vector.tensor_mul(lsel[:], finT[:], fview("lastsel", RL))
            lastT = cst.tile([H, NH], F32R, name="lastT")
            with nc.allow_low_precision(reason="f32r is fp32 bits"):
                nc.vector.reduce_sum(
                    lastT[:], lsel[:].rearrange("p (b l) -> p b l", b=BLOC),
                    axis=AX.X)

            # ---- batched multi-head attention (q = last, kv = final) ----
            qT = cst.tile([H, NH], F32, name="qT")
            ps_q = psum([H, NH], tag="ps")
            nc.tensor.matmul(ps_q[:], rview("prjT", 0), lastT[:])
            nc.scalar.activation(qT[:], ps_q[:], ACT.Identity,
                                 bias=fview("prjb"))
            kT = cst.tile([H, RL], F16, name="kT")
            ps_k = psum([H, RL], tag="ts")
            nc.tensor.matmul(ps_k[:], rview("prjT", H), finT[:])
            nc.scalar.activation(kT[:], ps_k[:], ACT.Identity,
                                 bias=packf[:, _OF_F["prjb"] + 1:
                                            _OF_F["prjb"] + 2])
            vT = cst.tile([H, RL], F16, name="vT")
            ps_v = psum([H, RL], tag="ts")
            nc.tensor.matmul(ps_v[:], rview("prjT", 2 * H), finT[:])
            nc.scalar.activation(vT[:], ps_v[:], ACT.Identity,
                                 bias=packf[:, _OF_F["prjb"] + 2:
                                            _OF_F["prjb"] + 3])

            qk = cst.tile([H, RL], F16, name="qk")
            nc.vector.tensor_mul(
                qk[:].rearrange("p (b l) -> p b l", b=BLOC),
                kT[:].rearrange("p (b l) -> p b l", b=BLOC),
                qT[:].to_broadcast([H, NH, L]))
            ps_att = psum([H, RL], tag="gg")
            nc.tensor.matmul(ps_att[:], hview("bd128"), qk[:])
            att2 = cst.tile([H, RL], F16, name="att2")
            nc.vector.tensor_add(att2[:], ps_att[:], fview("attm", RL))
            attE = cst.tile([H, RL], F16, name="attE")
            nc.scalar.activation(attE[:], att2[:], ACT.Exp)
            aden = cst.tile([H, NH], F32, name="aden")
            nc.vector.reduce_sum(
                aden[:], attE[:].rearrange("p (b l) -> p b l", b=BLOC),
                axis=AX.X)
            arec = cst.tile([H, NH], F32, name="arec")
            nc.vector.reciprocal(arec[:], aden[:])
            attw = cst.tile([H, RL], F16, name="attw")
            nc.vector.tensor_mul(
                attw[:].rearrange("p (b l) -> p b l", b=BLOC),
                attE[:].rearrange("p (b l) -> p b l", b=BLOC),
                arec[:].to_broadcast([H, NH, L]))
            pv = cst.tile([H, RL], F16, name="pv")
            nc.vector.tensor_mul(pv[:], attw[:], vT[:])
            ctxT = cst.tile([H, NH], F32R, name="ctxT")
            with nc.allow_low_precision(reason="f32r is fp32 bits"):
                nc.vector.reduce_sum(
                    ctxT[:], pv[:].rearrange("p (b l) -> p b l", b=BLOC),
                    axis=AX.X)

            sgloT = cst.tile([H, NH], F32, name="sgloT")
            ps_sg = psum([H, NH], tag="ps")
            nc.tensor.matmul(ps_sg[:], rview("oprjT"), ctxT[:])
            nc.scalar.activation(sgloT[:], ps_sg[:], ACT.Identity,
                                 bias=fview("oprjb"))

            # ---- u = w3_2 @ last + w3_3 @ sglo; t23 = cand @ u ----
            last16 = cst.tile([H, NH], F16, name="last16")
            nc.vector.tensor_copy(last16[:], lastT[:])
            sglo16 = cst.tile([H, NH], F16, name="sglo16")
            nc.vector.tensor_copy(sglo16[:], sgloT[:])
            ps_u = psum([H, NH], tag="ps")
            nc.tensor.matmul(ps_u[:], hview("w32T"), last16[:],
                             start=True, stop=False)
            nc.tensor.matmul(ps_u[:], hview("w33T"), sglo16[:],
                             start=False, stop=True)
            u16 = cst.tile([H, NH], F16, name="u16")
            nc.scalar.activation(u16[:], ps_u[:], ACT.Identity)

            # =======================================================
            # Phase D: target attention, 80 candidate chunks x 8 sessions.
            # One t23 piece (cand @ u) interleaved per group.
            # =======================================================
            out_all = cst.tile([H, NCH * BLOC], F32, name="out_all")
            dn_all = cst.tile([H, NCH // 4, 2, 4 * BLOC], F32, name="dn_all")
            NG = NCH // 4
            HG = NG // 2

            def npadb(ng):
                return fview("npadl", 32).to_broadcast(
                    [H, 32, ng]).rearrange("p b g -> p g b")

            def pair_tail(g0):
                """softmax tail for group pair (g0, g0+1), [H,64] ops."""
                a = dn_all[:, g0:g0 + 2, 0, :]
                b = dn_all[:, g0:g0 + 2, 1, :]
                osl = out_all[:, g0 * 32:(g0 + 2) * 32].rearrange(
                    "p (g b) -> p g b", g=2)
                dnf2 = wk.tile([H, 2, 4 * BLOC], F32, tag="dnf2", bufs=2)
                rc2 = wk.tile([H, 2, 4 * BLOC], F32, tag="rc2", bufs=2)
                if g0 < NGPM:  # E+/E- pair
                    s2 = wk.tile([H, 2, 4 * BLOC], F32, tag="s2", bufs=2)
                    nc.gpsimd.tensor_add(s2[:], a, b)
                    nc.vector.scalar_tensor_tensor(
                        out=dnf2[:], in0=s2[:], scalar=0.5, in1=npadb(2),
                        op0=ALU.mult, op1=ALU.subtract)
                    nc.vector.reciprocal(
                        rc2[:].rearrange("p g b -> p (g b)"),
                        dnf2[:].rearrange("p g b -> p (g b)"))
                    diff2 = wk.tile([H, 2, 4 * BLOC], F32, tag="df2", bufs=2)
                    nc.gpsimd.tensor_sub(diff2[:], a, b)
                    nc.vector.scalar_tensor_tensor(
                        out=osl, in0=diff2[:], scalar=1.0 / (2.0 * EPS),
                        in1=rc2[:], op0=ALU.mult, op1=ALU.mult)
                else:
                    nc.gpsimd.tensor_sub(dnf2[:], a, npadb(2))
                    nc.vector.reciprocal(
                        rc2[:].rearrange("p g b -> p (g b)"),
                        dnf2[:].rearrange("p g b -> p (g b)"))
                    nc.gpsimd.tensor_mul(osl, b, rc2[:])

            pend = []
            fdp2 = None
            for grp in range(NG):
                # E and E*g in ONE tile so the fold is a single GpSimd op
                # (+/- groups keep f32 to avoid cancellation noise)
                pm = grp < NGPM
                epT = wk.tile([H, 2, 4 * BLOC, L], F32 if pm else F16,
                              tag="epTP" if pm else "epT", bufs=2)
                for j in range(4):
                    ch = grp * 4 + j
                    js = slice(j * BLOC, (j + 1) * BLOC)
                    if grp < NGPM:
                        # E+/E- path: two exps, no elementwise product
                        ps_ts = psum([H, RL], tag="ts")
                        nc.tensor.matmul(ps_ts[:],
                                         trTp[:, ch * H:(ch + 1) * H],
                                         fin16[:])
                        ps_g = psum([H, RL], tag="gg")
                        nc.tensor.matmul(ps_g[:],
                                         trTm[:, ch * H:(ch + 1) * H],
                                         fin16[:])
                        nc.scalar.activation(
                            epT[:, 0, js, :].rearrange("p b l -> p (b l)"),
                            ps_ts[:], ACT.Exp)
                        nc.scalar.activation(
                            epT[:, 1, js, :].rearrange("p b l -> p (b l)"),
                            ps_g[:], ACT.Exp)
                        continue
                    hf, co = divmod((ch - NPM) * H, NPH)
                    ps_ts = psum([H, RL], tag="ts")
                    nc.tensor.matmul(ps_ts[:],
                                     trTh[hf][:, co:co + H], fin16[:])
                    ps_g = psum([H, RL], tag="gg")
                    nc.tensor.matmul(ps_g[:],
                                     c0h[hf][:, co:co + H], fin16[:])
                    nc.scalar.activation(
                        epT[:, 0, js, :].rearrange("p b l -> p (b l)"),
                        ps_ts[:], ACT.Exp)
                    nc.vector.tensor_mul(
                        epT[:, 1, js, :].rearrange("p b l -> p (b l)"),
                        epT[:, 0, js, :].rearrange("p b l -> p (b l)"),
                        ps_g[:])
                # t23 piece for this group (streamed cand + Scalar copy)
                cpi = wk.tile([H, 512], F16, tag="cpi", bufs=2)
                nc.sync.dma_start(cpi[:],
                                  candt_d[:, grp * 512:(grp + 1) * 512])
                ps_t23 = psum([NH, 512], tag="ps")
                nc.tensor.matmul(ps_t23[:], u16[:], cpi[:])
                t23s = wk.tile([NH, 512], F32, tag="t23s", bufs=2)
                nc.scalar.activation(t23s[:], ps_t23[:], ACT.Identity)
                nc.sync.dma_start(t23_out[:, grp * 512:(grp + 1) * 512],
                                  t23s[:])
                # fold 50->25 on GpSimd (one op per group into a 2-group
                # buffer); merged [128,128,25] reduce on Vector every two
                # groups, software-pipelined so it never blocks muls
                if grp % 2 == 0:
                    fdp2 = wk.tile([H, 2, 2 * 4 * BLOC, 25],
                                   F32 if pm else F16,
                                   tag="fdpP" if pm else "fdp", bufs=2)
                nc.gpsimd.tensor_add(
                    fdp2[:, grp % 2, :, :].rearrange("p a l -> p a l"),
                    epT[:, :, :, 0:25].rearrange("p a b l -> p (a b) l"),
                    epT[:, :, :, 25:50].rearrange("p a b l -> p (a b) l"))
                if grp == NG - 2:
                    # last pair split into single-group reduces so group
                    # NG-2's reduce hides under the final group's matmuls
                    nc.vector.reduce_sum(
                        dn_all[:, grp, :, :].rearrange("p a b -> p (a b)"),
                        fdp2[:, 0, :, :], axis=AX.X)
                elif grp % 2 == 1 and grp != NG - 1:
                    pend.append((grp - 1, fdp2))
                if len(pend) > 1 or (pend and grp == NG - 1):
                    g0, f0 = pend.pop(0)
                    nc.vector.reduce_sum(
                        dn_all[:, g0:g0 + 2, :, :].rearrange(
                            "p g a b -> p (g a b)"),
                        f0[:].rearrange("p g a l -> p (g a) l"), axis=AX.X)
                    if g0 >= 2:
                        pair_tail(g0 - 2)
            nc.vector.reduce_sum(
                dn_all[:, NG - 1, :, :].rearrange("p a b -> p (a b)"),
                fdp2[:, 1, :, :], axis=AX.X)
            pair_tail(NG - 4)
            pair_tail(NG - 2)
            nc.sync.dma_start(s1_out[:], out_all[:])

    nc.compile()
    return nc


# ==============================================================
# Host side: shard inputs, run, gather output
# ==============================================================

def _prep(inputs):
    """Build per-core input maps (numpy only: layout/sharding/index prep)."""
    emb = np.asarray(inputs["emb"], np.float32)
    items = np.asarray(inputs["session_items"], np.int32)
    lens = np.asarray(inputs["session_len"], np.int32)
    adj = np.asarray(inputs["session_adj"], np.float32)
    erow = np.asarray(inputs["global_edge_row"], np.int32)
    ecol_g = np.asarray(inputs["global_edge_col"], np.int32)
    ew_g = np.asarray(inputs["global_edge_weight"], np.float32)
    emb16 = emb.astype(np.float16)
    pos_emb = np.asarray(inputs["pos_emb"], np.float32)

    # ---- packed replicated constants ----
    packf = np.zeros((H, PF), np.float32)

    def setf(name, arr):
        o = _OF_F[name]
        arr = np.asarray(arr, np.float32)
        packf[:, o:o + (arr.shape[1] if arr.ndim > 1 else 1)] = (
            arr if arr.ndim > 1 else arr[:, None])

    packm0 = np.zeros((H, PM), np.float32)

    setf("blinrow", np.broadcast_to(
        np.asarray(inputs["lin_in_b"], np.float32)[None, :], (H, H)))
    setf("bloutrow", np.broadcast_to(
        np.asarray(inputs["lin_out_b"], np.float32)[None, :], (H, H)))
    setf("bih", np.asarray(inputs["b_ih"], np.float32).reshape(3, H).T)
    setf("bhh", np.asarray(inputs["b_hh"], np.float32).reshape(3, H).T)
    ipw = np.asarray(inputs["in_proj_w"], np.float32).copy()
    ipb = np.asarray(inputs["in_proj_b"], np.float32).copy()
    scale = 1.0 / math.sqrt(H // NH)
    ipw[:H] *= scale
    ipb[:H] *= scale
    setf("prjb", ipb.reshape(3, H).T)
    setf("oprjb", np.asarray(inputs["out_proj_b"], np.float32))
    setf("gb", np.asarray(inputs["gb"], np.float32))

    packr = np.zeros((H, PR), np.float32)

    def setr(name, arr):
        o = _OF_R[name]
        packr[:, o:o + arr.shape[1]] = arr

    setr("linT", np.asarray(inputs["lin_in_W"], np.float32).T)
    setr("loutT", np.asarray(inputs["lin_out_W"], np.float32).T)
    setr("whh", np.asarray(inputs["w_hh"], np.float32).T)
    setr("prjT", ipw.T)
    setr("oprjT", np.asarray(inputs["out_proj_w"], np.float32).T)
    wihT = np.asarray(inputs["w_ih"], np.float32).T  # [2H, 3H]
    setr("wih", wihT.reshape(2, H, 3 * H).transpose(1, 0, 2).reshape(H, 6 * H))

    # candidate-side transforms (host): cand = emb[1:], padded to NPAD
    cand_full = np.zeros((NPAD, H), np.float32)
    cand_full[:NIT - 1] = emb[1:]
    w3 = np.asarray(inputs["w3_W"], np.float32)           # [H, 3H]
    wt = np.asarray(inputs["w_target_W"], np.float32)     # [H, H]
    candT = cand_full.T                                    # [H, NPAD]
    trT_h = wt @ candT                                     # [H, NPAD]
    c0_h = w3[:, 0:H].T @ candT                            # [H, NPAD]

    packh = np.zeros((H, PH), np.float16)

    def seth(name, arr):
        o = _OF_H[name]
        packh[:, o:o + arr.shape[1]] = arr

    seth("w32T", w3[:, H:2 * H].T)
    seth("w33T", w3[:, 2 * H:3 * H].T)
    seth("gWT", np.asarray(inputs["gW"], np.float32).T)
    seth("bd128", np.kron(np.eye(NH, dtype=np.float32),
                          np.ones((H // NH, H // NH), np.float32)))

    trT16 = trT_h.astype(np.float16)
    c016 = c0_h.astype(np.float16)
    npm = NPM * H
    trTd = trT16[:, NPM * H:]
    c0d = c016[:, NPM * H:]
    rep = dict(packr=packr, packh=packh,
               trTa=trTd[:, :NPH].copy(), trTb=trTd[:, NPH:].copy(),
               c0a=c0d[:, :NPH].copy(), c0b=c0d[:, NPH:].copy(),
               trTp=(trT_h[:, :npm] + EPS * c0_h[:, :npm]).astype(np.float16),
               trTm=(trT_h[:, :npm] - EPS * c0_h[:, :npm]).astype(np.float16),
               candt=candT.astype(np.float16))

    # --- global edges: route to each core's session position slots ---
    order = np.argsort(erow, kind="stable")
    erow_s, ecol_s, ew_s = erow[order], ecol_g[order], ew_g[order]
    rstart = np.searchsorted(erow_s, np.arange(NIT + 1)).astype(np.int64)

    core_pos = []
    maxwin = 0
    for c in range(NC):
        it_flat = items[c * BLOC:(c + 1) * BLOC].reshape(-1).astype(np.int64)
        cnts = np.where(it_flat == 0, 0, rstart[it_flat + 1] - rstart[it_flat])
        wcnt = cnts.reshape(NWINA, WINA).sum(1)
        maxwin = max(maxwin, int(wcnt.max()))
        core_pos.append((it_flat, cnts, wcnt))
    T = max(1, int(math.ceil(maxwin / H)))
    NT = NWINA * T

    per_core = []
    for c in range(NC):
        it_flat, cnts, wcnt = core_pos[c]
        total = int(cnts.sum())
        starts_pos = rstart[it_flat]
        excl = np.cumsum(cnts) - cnts  # exclusive prefix
        src = np.repeat(starts_pos - excl, cnts) + np.arange(total)
        pos_rep = np.repeat(np.arange(RL), cnts)

        ec = np.zeros((NWINA, T * H), np.int32)
        er = np.full((NWINA, T * H), 300.0, np.float32)
        evw = np.zeros((NWINA, T * H), np.float32)
        wb = np.zeros(NWINA + 1, np.int64)
        np.cumsum(wcnt, out=wb[1:])
        for w in range(NWINA):
            s, e = wb[w], wb[w + 1]
            n = e - s
            ec[w, :n] = ecol_s[src[s:e]]
            er[w, :n] = (pos_rep[s:e] - w * WINA).astype(np.float32)
            evw[w, :n] = ew_s[src[s:e]]
        ec2 = ec.reshape(NT, H).T
        er2 = er.reshape(NT, H).T
        ev2 = evw.reshape(NT, H).T
        sw = ((er2[:, :, None] == np.arange(WINA, dtype=np.float32)) *
              ev2[:, :, None]).astype(np.float16)

        bsl = slice(c * BLOC, (c + 1) * BLOC)
        it_loc = items[bsl]                      # [8, 50]
        len_loc = lens[bsl]
        pos_idx = np.arange(L)[None, :]
        rev = len_loc[:, None] - 1 - pos_idx
        rev = np.where(it_loc == 0, 0, rev).astype(np.int32)
        pad = (it_loc == 0)

        h0x = np.ascontiguousarray(emb[it_loc.reshape(-1)].T)
        pox = np.ascontiguousarray(pos_emb[rev.reshape(-1)].T)

        pm_c = packm0.copy()
        attm = np.where(pad, -30000.0, 0.0).astype(np.float32).reshape(1, RL)
        pm_c[:, _OF_M["attm"]:_OF_M["attm"] + RL] = attm
        colmask = (~pad).astype(np.float32).reshape(1, RL)
        pm_c[:, _OF_M["colm"]:_OF_M["colm"] + RL] = colmask
        lastsel = np.zeros((BLOC, L), np.float32)
        lastsel[np.arange(BLOC), len_loc - 1] = 1.0
        pm_c[:, _OF_M["lastsel"]:_OF_M["lastsel"] + RL] = lastsel.reshape(
            1, RL)
        npadl = np.tile((L - len_loc).astype(np.float32), 4)  # [32]
        pm_c[:, _OF_M["npadl"]:_OF_M["npadl"] + 32] = npadl[None, :]

        adjbd = np.zeros((BLOC // 2, 2 * L, 2 * L), np.float32)
        for j in range(BLOC // 2):
            for i in range(2):
                adjbd[j, i * L:(i + 1) * L, i * L:(i + 1) * L] = (
                    adj[c * BLOC + 2 * j + i].T)

        m = dict(rep)
        m["packf"] = packf
        m["packm"] = pm_c
        m["h0x"] = h0x
        m["pox"] = pox
        m["adjbd"] = adjbd
        m["eemb"] = np.ascontiguousarray(emb16[ec2])
        m["swt"] = np.ascontiguousarray(sw)
        per_core.append(m)
    return per_core, T


def kernel(_trace=False, **inputs):
    in_maps, T = _prep(inputs)
    if T not in _NC_CACHE:
        _NC_CACHE[T] = build_nc(T)
    nc = _NC_CACHE[T]
    res = run_bass_kernel_spmd(nc, in_maps, core_ids=list(range(NC)),
                               trace=_trace)
    rows = []
    for c in range(NC):
        s1 = res.results[c]["scores1"].reshape(H, NCH, BLOC)
        s1 = s1.transpose(2, 1, 0).reshape(BLOC, NPAD)
        rows.append(s1 + res.results[c]["t23"])
    scores = np.concatenate(rows, axis=0)[:, :NIT - 1]
    if _trace:
        return scores, res
    return scores


# revision 67
# speedup vs baseline: 1.0172x; 1.0172x over previous
"""Trainium2 Bass kernel for GCE-TAGNN session recommendation model.

Strategy: batch-sharded, collective-free.
  - Each core owns 8 sessions and scores them against ALL 10240 (padded)
    candidates: no all-gather, no barrier, no launch-skew sensitivity.
  - Global GNN: hg is only consumed as hg[session_items], so each core
    aggregates ONLY the edges targeting its own sessions' items (host-routed
    per position slot) and applies gW/relu locally -> sess_glob directly.
  - Session adjacency matmuls are transpose-free: Y^T computed directly via
    matmul with h0T as weights, then block-diagonal (2 sessions) adj matmul.
  - MHA batched across all 8 local sessions using a head-replicated
    block-diagonal matmul; softmax pipeline runs on [128, 400] tiles.
  - Target attention: with d = cand @ w3_W ([N,384]),
      scores[b,n] = (sum_l E*g)/(sum_l E) + last[b]*d[n,128:256]
                    + s_global[b]*d[n,256:384]
    ts = final·(w_target_W cand[n]), E = exp(ts) (|ts| tiny, no max needed),
    g = final·d[n,:128].  trT/c0 transforms precomputed on host (fp16).
    last/sglo terms = cand[n]·u_b with u = w3_2 last + w3_3 sglo: emitted as
    20 wide [8,512] matmuls in [b, n] layout; the HOST adds them to the
    device's [n, b] softmax term (free transpose-avoidance).
    Per-b softmax denominator corrected by subtracting (L - len[b]).
    The first 24 candidate chunks use a finite-difference trick to avoid the
    elementwise E*g product entirely (balancing Vector vs Scalar):
      sum_l E*g = (sum_l exp(ts+eps*g) - sum_l exp(ts-eps*g)) / (2*eps)
    with host-precombined weight tables trT +- eps*c0 and fp32 accumulation.
    Engine split in phase D: exp on Scalar, E*g on Vector, 50->25 folds on
    GpSimd, 25-col segment reductions on Vector, softmax tails half-hidden
    under the main loop.
"""

import sys

sys.path.insert(0, "/opt/trn_rl_repo")

import math

import numpy as np

import concourse.mybir as mybir
import concourse.tile as tile
from concourse import bacc
from concourse.bass_utils import run_bass_kernel_spmd

F32 = mybir.dt.float32
F32R = mybir.dt.float32r
F16 = mybir.dt.float16
I32 = mybir.dt.int32
AX = mybir.AxisListType
ALU = mybir.AluOpType
ACT = mybir.ActivationFunctionType

NC = 8          # cores
B = 64          # batch
L = 50          # session length
H = 128         # hidden
NH = 8          # heads
NIT = 10000     # item vocab
NPAD = 10240    # padded vocab
NCH = NPAD // H  # 80 candidate chunks of 128
BLOC = B // NC  # sessions per core
RL = BLOC * L   # 400 position slots per core
WINA = 16       # agg position window
NWINA = RL // WINA  # 25 windows per core

# ---- packed-constant column offsets ----
_OF_F = {}
_o = 0
for _n, _w in [("blinrow", H), ("bloutrow", H),
               ("bih", 3), ("bhh", 3), ("prjb", 3), ("oprjb", 1), ("gb", 1)]:
    _OF_F[_n] = _o
    _o += _w
PF = _o

_OF_M = {}
_o = 0
for _n, _w in [("attm", RL), ("colm", RL), ("lastsel", RL), ("npadl", 32)]:
    _OF_M[_n] = _o
    _o += _w
PM = _o

_OF_R = {}
_o = 0
for _n, _w in [("linT", H), ("loutT", H), ("whh", 3 * H), ("prjT", 3 * H),
               ("oprjT", H), ("wih", 6 * H)]:
    _OF_R[_n] = _o
    _o += _w
PR = _o

_OF_H = {}
_o = 0
for _n, _w in [("w32T", H), ("w33T", H), ("gWT", H), ("bd128", H)]:
    _OF_H[_n] = _o
    _o += _w
PH = _o

NGPM = 8         # leading D groups using the +/-eps finite-difference path
NPM = NGPM * 4   # chunks on that path
NDIR = NPAD - NPM * H   # direct-path candidate columns
NPH = NDIR // 2  # direct-path half-width for priority-ordered uploads
EPS = 2.4        # finite-difference step: E*g = (E+ - E-)/(2*EPS)

_NC_CACHE = {}


def build_nc(T):
    """Build the per-core program. T = edge tiles per position window."""
    NT = NWINA * T  # edge tiles per core
    nc = bacc.Bacc(None, target_bir_lowering=False)

    def inp(name, shape, dtype=F32):
        return nc.dram_tensor(name, shape, dtype, kind="ExternalInput")

    h0x_d = inp("h0x", [H, RL], F32R)   # emb[items]^T, host-gathered
    pox_d = inp("pox", [H, RL])         # pos_emb[rev]^T, host-gathered
    packf_d = inp("packf", [H, PF])
    packm_d = inp("packm", [H, PM])
    packr_d = inp("packr", [H, PR], F32R)
    packh_d = inp("packh", [H, PH], F16)
    trTa_d = inp("trTa", [H, NPH], F16)
    trTb_d = inp("trTb", [H, NPH], F16)
    c0a_d = inp("c0a", [H, NPH], F16)
    c0b_d = inp("c0b", [H, NPH], F16)
    candt_d = inp("candt", [H, NPAD], F16)
    trTp_d = inp("trTp", [H, NPM * H], F16)
    trTm_d = inp("trTm", [H, NPM * H], F16)
    adjbd_d = inp("adjbd", [BLOC // 2, 2 * L, 2 * L], F32R)
    eemb = inp("eemb", [H, NT, H], F16)
    swt = inp("swt", [H, NT, WINA], F16)

    s1_out = nc.dram_tensor("scores1", [H, NCH * BLOC], F32,
                            kind="ExternalOutput")
    t23_out = nc.dram_tensor("t23", [NH, NPAD], F32, kind="ExternalOutput")

    with tile.TileContext(nc) as tc:
        with (
            tc.tile_pool(name="cst", bufs=1) as cst,
            tc.tile_pool(name="wk", bufs=3) as wk,
            tc.tile_pool(name="pp", bufs=8, space="PSUM") as pp,
        ):
            def psum(shape, tag="ps", dtype=F32):
                nbuf = {"ps": 2, "ts": 3, "gg": 3}[tag]
                return pp.tile(shape, dtype, tag=tag, name=tag, bufs=nbuf)

            # ---------- constant loads (packed); big packh goes LAST so
            # phase A/B inputs aren't queued behind it ----------
            h0T = cst.tile([H, RL], F32R, name="h0T")
            nc.sync.dma_start(h0T[:], h0x_d[:])
            adjbd = cst.tile([2 * L, BLOC // 2, 2 * L], F32R, name="adjbd")
            nc.sync.dma_start(adjbd[:], adjbd_d.rearrange("j p k -> p j k"))
            packf = cst.tile([H, PF], F32, name="packf")
            nc.sync.dma_start(packf[:], packf_d[:])
            packr = cst.tile([H, PR], F32R, name="packr")
            nc.sync.dma_start(packr[:], packr_d[:])
            emA = wk.tile([H, NT, H], F16, tag="epTP", bufs=2)
            nc.sync.dma_start(emA[:], eemb[:])
            swA = wk.tile([H, NT, WINA], F16, tag="epTP", bufs=2)
            nc.sync.dma_start(swA[:], swt[:])
            poT = cst.tile([H, RL], F32, name="poT")
            nc.sync.dma_start(poT[:], pox_d[:])
            packm = cst.tile([H, PM], F32, name="packm")
            nc.sync.dma_start(packm[:], packm_d[:])
            packh = cst.tile([H, PH], F16, name="packh")
            nc.sync.dma_start(packh[:], packh_d[:])
            # candidate-side transforms, priority-ordered: first halves of
            # trT/c0 land first so phase D can start before the rest arrive
            trTh = [cst.tile([H, NPH], F16, name=f"trT{i}") for i in range(2)]
            c0h = [cst.tile([H, NPH], F16, name=f"c0{i}") for i in range(2)]
            trTp = cst.tile([H, NPM * H], F16, name="trTp")
            nc.sync.dma_start(trTp[:], trTp_d[:])
            trTm = cst.tile([H, NPM * H], F16, name="trTm")
            nc.sync.dma_start(trTm[:], trTm_d[:])
            nc.sync.dma_start(trTh[0][:], trTa_d[:])
            nc.sync.dma_start(c0h[0][:], c0a_d[:])
            nc.sync.dma_start(trTh[1][:], trTb_d[:])
            nc.sync.dma_start(c0h[1][:], c0b_d[:])


            def fview(name, w=None):
                if name in _OF_M:
                    o = _OF_M[name]
                    return packm[:, o:o + (w if w is not None else 1)]
                o = _OF_F[name]
                return packf[:, o:o + (w if w is not None else 1)]

            def rview(name, off=0, w=H):
                return packr[:, _OF_R[name] + off:_OF_R[name] + off + w]

            def hview(name, off=0, w=H):
                return packh[:, _OF_H[name] + off:_OF_H[name] + off + w]

            # =======================================================
            # Phase B: session path (8 local sessions)
            # inp = adj @ (h W^T + b) via transpose-free block-diag matmuls
            # =======================================================
            iinT = cst.tile([H, RL], F32R, name="iinT")
            ioutT = cst.tile([H, RL], F32R, name="ioutT")
            for blk in range(4):
                sl = slice(blk * 2 * L, (blk + 1) * 2 * L)
                for wname, brow, dst in (("linT", "blinrow", iinT),
                                         ("loutT", "bloutrow", ioutT)):
                    ps_yt = psum([2 * L, H], tag="ps")
                    nc.tensor.matmul(ps_yt[:], h0T[:, sl], rview(wname))
                    yt = wk.tile([2 * L, H], F32R, tag="yt")
                    nc.vector.tensor_add(yt[:], ps_yt[:],
                                         packf[0:2 * L,
                                               _OF_F[brow]:_OF_F[brow] + H])
                    ps_ii = psum([H, 2 * L], tag="gg")
                    nc.tensor.matmul(ps_ii[:], yt[:], adjbd[:, blk, :])
                    nc.vector.tensor_copy(dst[:, sl], ps_ii[:])

            # =======================================================
            # Phase A: GNN aggregation for this core's session positions.
            # =======================================================
            agg_ps = psum([H, RL], tag="ts")
            for w in range(NWINA):
                for t in range(T):
                    j = w * T + t
                    nc.tensor.matmul(
                        agg_ps[:, w * WINA:(w + 1) * WINA],
                        emA[:, j, :], swA[:, j, :],
                        start=(t == 0), stop=(t == T - 1))
            aggA = cst.tile([H, RL], F16, name="aggA")
            nc.vector.tensor_copy(aggA[:], agg_ps[:])
            # sess_glob^T = relu(gW @ agg + gb) in position order
            sgA = cst.tile([H, RL], F32, name="sgA")
            ps_sga = psum([H, RL], tag="gg")
            nc.tensor.matmul(ps_sga[:], hview("gWT"), aggA[:])
            nc.scalar.activation(sgA[:], ps_sga[:], ACT.Relu,
                                 bias=fview("gb"))

            # GRU cell (feature-major)
            combR = cst.tile([H, 2], F32, name="combR")
            nc.vector.tensor_add(combR[:, 0:1], fview("bih"), fview("bhh"))
            nc.vector.tensor_add(combR[:, 1:2],
                                 packf[:, _OF_F["bih"] + 1:_OF_F["bih"] + 2],
                                 packf[:, _OF_F["bhh"] + 1:_OF_F["bhh"] + 2])
            gates = []
            for g in range(2):  # r, z
                ps_gate = psum([H, RL], tag="ts")
                nc.tensor.matmul(ps_gate[:], rview("wih", g * H),
                                 iinT[:], start=True, stop=False)
                nc.tensor.matmul(ps_gate[:], rview("wih", 3 * H + g * H),
                                 ioutT[:], start=False, stop=False)
                nc.tensor.matmul(ps_gate[:], rview("whh", g * H),
                                 h0T[:], start=False, stop=True)
                gt = cst.tile([H, RL], F32, name=f"gate{g}")
                nc.scalar.activation(gt[:], ps_gate[:], ACT.Sigmoid,
                                     bias=combR[:, g:g + 1])
                gates.append(gt)
            rT, zT = gates
            ps_in = psum([H, RL], tag="ts")
            nc.tensor.matmul(ps_in[:], rview("wih", 2 * H), iinT[:],
                             start=True, stop=False)
            nc.tensor.matmul(ps_in[:], rview("wih", 5 * H), ioutT[:],
                             start=False, stop=True)
            ps_hn = psum([H, RL], tag="gg")
            nc.tensor.matmul(ps_hn[:], rview("whh", 2 * H), h0T[:])
            rhn = cst.tile([H, RL], F32, name="rhn")
            nc.vector.scalar_tensor_tensor(
                out=rhn[:], in0=ps_hn[:],
                scalar=packf[:, _OF_F["bhh"] + 2:_OF_F["bhh"] + 3],
                in1=rT[:], op0=ALU.add, op1=ALU.mult)
            tmp_n = cst.tile([H, RL], F32, name="tmp_n")
            nc.vector.tensor_add(tmp_n[:], ps_in[:], rhn[:])
            nT = cst.tile([H, RL], F32, name="nT")
            nc.scalar.activation(nT[:], tmp_n[:], ACT.Tanh,
                                 bias=packf[:, _OF_F["bih"] + 2:
                                            _OF_F["bih"] + 3])
            diff = cst.tile([H, RL], F32, name="diff")
            nc.vector.tensor_sub(diff[:], h0T[:], nT[:])
            zd = cst.tile([H, RL], F32, name="zd")
            nc.vector.tensor_mul(zd[:], zT[:], diff[:])
            h1T = cst.tile([H, RL], F32, name="h1T")
            nc.vector.tensor_add(h1T[:], nT[:], zd[:])

            # rich = sess_glob + h1; final = (rich + pos_emb[rev]) * colmask
            richT = cst.tile([H, RL], F32, name="richT")
            nc.vector.tensor_add(richT[:], h1T[:], sgA[:])
            finT = cst.tile([H, RL], F32R, name="finT")
            nc.vector.tensor_add(finT[:], richT[:], poT[:])
            nc.vector.tensor_mul(finT[:], finT[:], fview("colm", RL))
            fin16 = cst.tile([H, RL], F16, name="fin16")
            nc.vector.tensor_copy(fin16[:], finT[:])

            # last[b] = final[b, len_b - 1]  (one-hot selection + reduce)
            lsel = cst.tile([H, RL], F32, name="lsel")
            nc.vector.tensor_mul(lsel[:], finT[:], fview("lastsel", RL))
            lastT = cst.tile([H, NH], F32R, name="lastT")
            with nc.allow_low_precision(reason="f32r is fp32 bits"):
                nc.vector.reduce_sum(
                    lastT[:], lsel[:].rearrange("p (b l) -> p b l", b=BLOC),
                    axis=AX.X)

            # ---- batched multi-head attention (q = last, kv = final) ----
            qT = cst.tile([H, NH], F32, name="qT")
            ps_q = psum([H, NH], tag="ps")
            nc.tensor.matmul(ps_q[:], rview("prjT", 0), lastT[:])
            nc.scalar.activation(qT[:], ps_q[:], ACT.Identity,
                                 bias=fview("prjb"))
            kT = cst.tile([H, RL], F16, name="kT")
            ps_k = psum([H, RL], tag="ts")
            nc.tensor.matmul(ps_k[:], rview("prjT", H), finT[:])
            nc.scalar.activation(kT[:], ps_k[:], ACT.Identity,
                                 bias=packf[:, _OF_F["prjb"] + 1:
                                            _OF_F["prjb"] + 2])
            vT = cst.tile([H, RL], F16, name="vT")
            ps_v = psum([H, RL], tag="ts")
            nc.tensor.matmul(ps_v[:], rview("prjT", 2 * H), finT[:])
            nc.scalar.activation(vT[:], ps_v[:], ACT.Identity,
                                 bias=packf[:, _OF_F["prjb"] + 2:
                                            _OF_F["prjb"] + 3])

            qk = cst.tile([H, RL], F16, name="qk")
            nc.vector.tensor_mul(
                qk[:].rearrange("p (b l) -> p b l", b=BLOC),
                kT[:].rearrange("p (b l) -> p b l", b=BLOC),
                qT[:].to_broadcast([H, NH, L]))
            ps_att = psum([H, RL], tag="gg")
            nc.tensor.matmul(ps_att[:], hview("bd128"), qk[:])
            att2 = cst.tile([H, RL], F16, name="att2")
            nc.vector.tensor_add(att2[:], ps_att[:], fview("attm", RL))
            attE = cst.tile([H, RL], F16, name="attE")
            nc.scalar.activation(attE[:], att2[:], ACT.Exp)
            aden = cst.tile([H, NH], F32, name="aden")
            nc.vector.reduce_sum(
                aden[:], attE[:].rearrange("p (b l) -> p b l", b=BLOC),
                axis=AX.X)
            arec = cst.tile([H, NH], F32, name="arec")
            nc.vector.reciprocal(arec[:], aden[:])
            attw = cst.tile([H, RL], F16, name="attw")
            nc.vector.tensor_mul(
                attw[:].rearrange("p (b l) -> p b l", b=BLOC),
                attE[:].rearrange("p (b l) -> p b l", b=BLOC),
                arec[:].to_broadcast([H, NH, L]))
            pv = cst.tile([H, RL], F16, name="pv")
            nc.vector.tensor_mul(pv[:], attw[:], vT[:])
            ctxT = cst.tile([H, NH], F32R, name="ctxT")
            with nc.allow_low_precision(reason="f32r is fp32 bits"):
                nc.vector.reduce_sum(
                    ctxT[:], pv[:].rearrange("p (b l) -> p b l", b=BLOC),
                    axis=AX.X)

            sgloT = cst.tile([H, NH], F32, name="sgloT")
            ps_sg = psum([H, NH], tag="ps")
            nc.tensor.matmul(ps_sg[:], rview("oprjT"), ctxT[:])
            nc.scalar.activation(sgloT[:], ps_sg[:], ACT.Identity,
                                 bias=fview("oprjb"))

            # ---- u = w3_2 @ last + w3_3 @ sglo; t23 = cand @ u ----
            last16 = cst.tile([H, NH], F16, name="last16")
            nc.vector.tensor_copy(last16[:], lastT[:])
            sglo16 = cst.tile([H, NH], F16, name="sglo16")
            nc.vector.tensor_copy(sglo16[:], sgloT[:])
            ps_u = psum([H, NH], tag="ps")
            nc.tensor.matmul(ps_u[:], hview("w32T"), last16[:],
                             start=True, stop=False)
            nc.tensor.matmul(ps_u[:], hview("w33T"), sglo16[:],
                             start=False, stop=True)
            u16 = cst.tile([H, NH], F16, name="u16")
            nc.scalar.activation(u16[:], ps_u[:], ACT.Identity)

            # =======================================================
            # Phase D: target attention, 80 candidate chunks x 8 sessions.
            # One t23 piece (cand @ u) interleaved per group.
            # =======================================================
            out_all = cst.tile([H, NCH * BLOC], F32, name="out_all")
            dn_all = cst.tile([H, NCH // 4, 2, 4 * BLOC], F32, name="dn_all")
            NG = NCH // 4
            HG = NG // 2

            def npadb(ng):
                return fview("npadl", 32).to_broadcast(
                    [H, 32, ng]).rearrange("p b g -> p g b")

            def pair_tail(g0):
                """softmax tail for group pair (g0, g0+1), [H,64] ops."""
                a = dn_all[:, g0:g0 + 2, 0, :]
                b = dn_all[:, g0:g0 + 2, 1, :]
                osl = out_all[:, g0 * 32:(g0 + 2) * 32].rearrange(
                    "p (g b) -> p g b", g=2)
                dnf2 = wk.tile([H, 2, 4 * BLOC], F32, tag="dnf2", bufs=2)
                rc2 = wk.tile([H, 2, 4 * BLOC], F32, tag="rc2", bufs=2)
                if g0 < NGPM:  # E+/E- pair
                    s2 = wk.tile([H, 2, 4 * BLOC], F32, tag="s2", bufs=2)
                    nc.gpsimd.tensor_add(s2[:], a, b)
                    nc.vector.scalar_tensor_tensor(
                        out=dnf2[:], in0=s2[:], scalar=0.5, in1=npadb(2),
                        op0=ALU.mult, op1=ALU.subtract)
                    nc.vector.reciprocal(
                        rc2[:].rearrange("p g b -> p (g b)"),
                        dnf2[:].rearrange("p g b -> p (g b)"))
                    diff2 = wk.tile([H, 2, 4 * BLOC], F32, tag="df2", bufs=2)
                    nc.gpsimd.tensor_sub(diff2[:], a, b)
                    nc.vector.scalar_tensor_tensor(
                        out=osl, in0=diff2[:], scalar=1.0 / (2.0 * EPS),
                        in1=rc2[:], op0=ALU.mult, op1=ALU.mult)
                else:
                    nc.gpsimd.tensor_sub(dnf2[:], a, npadb(2))
                    nc.vector.reciprocal(
                        rc2[:].rearrange("p g b -> p (g b)"),
                        dnf2[:].rearrange("p g b -> p (g b)"))
                    nc.gpsimd.tensor_mul(osl, b, rc2[:])

            pend = []
            fdp2 = None
            for grp in range(NG):
                # E and E*g in ONE tile so the fold is a single GpSimd op
                # (+/- groups keep f32 to avoid cancellation noise)
                pm = grp < NGPM
                epT = wk.tile([H, 2, 4 * BLOC, L], F32 if pm else F16,
                              tag="epTP" if pm else "epT", bufs=2)
                for j in range(4):
                    ch = grp * 4 + j
                    js = slice(j * BLOC, (j + 1) * BLOC)
                    if grp < NGPM:
                        # E+/E- path: two exps, no elementwise product
                        ps_ts = psum([H, RL], tag="ts")
                        nc.tensor.matmul(ps_ts[:],
                                         trTp[:, ch * H:(ch + 1) * H],
                                         fin16[:])
                        ps_g = psum([H, RL], tag="gg")
                        nc.tensor.matmul(ps_g[:],
                                         trTm[:, ch * H:(ch + 1) * H],
                                         fin16[:])
                        nc.scalar.activation(
                            epT[:, 0, js, :].rearrange("p b l -> p (b l)"),
                            ps_ts[:], ACT.Exp)
                        nc.scalar.activation(
                            epT[:, 1, js, :].rearrange("p b l -> p (b l)"),
                            ps_g[:], ACT.Exp)
                        continue
                    hf, co = divmod((ch - NPM) * H, NPH)
                    ps_ts = psum([H, RL], tag="ts")
                    nc.tensor.matmul(ps_ts[:],
                                     trTh[hf][:, co:co + H], fin16[:])
                    ps_g = psum([H, RL], tag="gg")
                    nc.tensor.matmul(ps_g[:],
                                     c0h[hf][:, co:co + H], fin16[:])
                    nc.scalar.activation(
                        epT[:, 0, js, :].rearrange("p b l -> p (b l)"),
                        ps_ts[:], ACT.Exp)
                    nc.vector.tensor_mul(
                        epT[:, 1, js, :].rearrange("p b l -> p (b l)"),
                        epT[:, 0, js, :].rearrange("p b l -> p (b l)"),
                        ps_g[:])
                # t23 piece for this group (streamed cand + Scalar copy)
                cpi = wk.tile([H, 512], F16, tag="cpi", bufs=2)
                nc.sync.dma_start(cpi[:],
                                  candt_d[:, grp * 512:(grp + 1) * 512])
                ps_t23 = psum([NH, 512], tag="ps")
                nc.tensor.matmul(ps_t23[:], u16[:], cpi[:])
                t23s = wk.tile([NH, 512], F32, tag="t23s", bufs=2)
                nc.scalar.activation(t23s[:], ps_t23[:], ACT.Identity)
                nc.sync.dma_start(t23_out[:, grp * 512:(grp + 1) * 512],
                                  t23s[:])
                # fold 50->25 on GpSimd (one op per group into a 2-group
                # buffer); merged [128,128,25] reduce on Vector every two
                # groups, software-pipelined so it never blocks muls
                if grp % 2 == 0:
                    fdp2 = wk.tile([H, 2, 2 * 4 * BLOC, 25],
                                   F32 if pm else F16,
                                   tag="fdpP" if pm else "fdp", bufs=2)
                nc.gpsimd.tensor_add(
                    fdp2[:, grp % 2, :, :].rearrange("p a l -> p a l"),
                    epT[:, :, :, 0:25].rearrange("p a b l -> p (a b) l"),
                    epT[:, :, :, 25:50].rearrange("p a b l -> p (a b) l"))
                if grp % 2 == 1:
                    pend.append((grp - 1, fdp2))
                if len(pend) > 1:
                    g0, f0 = pend.pop(0)
                    nc.vector.reduce_sum(
                        dn_all[:, g0:g0 + 2, :, :].rearrange(
                            "p g a b -> p (g a b)"),
                        f0[:].rearrange("p g a l -> p (g a) l"), axis=AX.X)
                    if g0 >= 2:
                        pair_tail(g0 - 2)
            g0, f0 = pend.pop(0)
            nc.vector.reduce_sum(
                dn_all[:, g0:g0 + 2, :, :].rearrange("p g a b -> p (g a b)"),
                f0[:].rearrange("p g a l -> p (g a) l"), axis=AX.X)
            pair_tail(g0 - 2)
            pair_tail(g0)
            nc.sync.dma_start(s1_out[:], out_all[:])

    nc.compile()
    return nc


# ==============================================================
# Host side: shard inputs, run, gather output
# ==============================================================

def _prep(inputs):
    """Build per-core input maps (numpy only: layout/sharding/index prep)."""
    emb = np.asarray(inputs["emb"], np.float32)
    items = np.asarray(inputs["session_items"], np.int32)
    lens = np.asarray(inputs["session_len"], np.int32)
    adj = np.asarray(inputs["session_adj"], np.float32)
    erow = np.asarray(inputs["global_edge_row"], np.int32)
    ecol_g = np.asarray(inputs["global_edge_col"], np.int32)
    ew_g = np.asarray(inputs["global_edge_weight"], np.float32)
    emb16 = emb.astype(np.float16)
    pos_emb = np.asarray(inputs["pos_emb"], np.float32)

    # ---- packed replicated constants ----
    packf = np.zeros((H, PF), np.float32)

    def setf(name, arr):
        o = _OF_F[name]
        arr = np.asarray(arr, np.float32)
        packf[:, o:o + (arr.shape[1] if arr.ndim > 1 else 1)] = (
            arr if arr.ndim > 1 else arr[:, None])

    packm0 = np.zeros((H, PM), np.float32)

    setf("blinrow", np.broadcast_to(
        np.asarray(inputs["lin_in_b"], np.float32)[None, :], (H, H)))
    setf("bloutrow", np.broadcast_to(
        np.asarray(inputs["lin_out_b"], np.float32)[None, :], (H, H)))
    setf("bih", np.asarray(inputs["b_ih"], np.float32).reshape(3, H).T)
    setf("bhh", np.asarray(inputs["b_hh"], np.float32).reshape(3, H).T)
    ipw = np.asarray(inputs["in_proj_w"], np.float32).copy()
    ipb = np.asarray(inputs["in_proj_b"], np.float32).copy()
    scale = 1.0 / math.sqrt(H // NH)
    ipw[:H] *= scale
    ipb[:H] *= scale
    setf("prjb", ipb.reshape(3, H).T)
    setf("oprjb", np.asarray(inputs["out_proj_b"], np.float32))
    setf("gb", np.asarray(inputs["gb"], np.float32))

    packr = np.zeros((H, PR), np.float32)

    def setr(name, arr):
        o = _OF_R[name]
        packr[:, o:o + arr.shape[1]] = arr

    setr("linT", np.asarray(inputs["lin_in_W"], np.float32).T)
    setr("loutT", np.asarray(inputs["lin_out_W"], np.float32).T)
    setr("whh", np.asarray(inputs["w_hh"], np.float32).T)
    setr("prjT", ipw.T)
    setr("oprjT", np.asarray(inputs["out_proj_w"], np.float32).T)
    wihT = np.asarray(inputs["w_ih"], np.float32).T  # [2H, 3H]
    setr("wih", wihT.reshape(2, H, 3 * H).transpose(1, 0, 2).reshape(H, 6 * H))

    # candidate-side transforms (host): cand = emb[1:], padded to NPAD
    cand_full = np.zeros((NPAD, H), np.float32)
    cand_full[:NIT - 1] = emb[1:]
    w3 = np.asarray(inputs["w3_W"], np.float32)           # [H, 3H]
    wt = np.asarray(inputs["w_target_W"], np.float32)     # [H, H]
    candT = cand_full.T                                    # [H, NPAD]
    trT_h = wt @ candT                                     # [H, NPAD]
    c0_h = w3[:, 0:H].T @ candT                            # [H, NPAD]

    packh = np.zeros((H, PH), np.float16)

    def seth(name, arr):
        o = _OF_H[name]
        packh[:, o:o + arr.shape[1]] = arr

    seth("w32T", w3[:, H:2 * H].T)
    seth("w33T", w3[:, 2 * H:3 * H].T)
    seth("gWT", np.asarray(inputs["gW"], np.float32).T)
    seth("bd128", np.kron(np.eye(NH, dtype=np.float32),
                          np.ones((H // NH, H // NH), np.float32)))

    trT16 = trT_h.astype(np.float16)
    c016 = c0_h.astype(np.float16)
    npm = NPM * H
    trTd = trT16[:, NPM * H:]
    c0d = c016[:, NPM * H:]
    rep = dict(packr=packr, packh=packh,
               trTa=trTd[:, :NPH].copy(), trTb=trTd[:, NPH:].copy(),
               c0a=c0d[:, :NPH].copy(), c0b=c0d[:, NPH:].copy(),
               trTp=(trT_h[:, :npm] + EPS * c0_h[:, :npm]).astype(np.float16),
               trTm=(trT_h[:, :npm] - EPS * c0_h[:, :npm]).astype(np.float16),
               candt=candT.astype(np.float16))

    # --- global edges: route to each core's session position slots ---
    order = np.argsort(erow, kind="stable")
    erow_s, ecol_s, ew_s = erow[order], ecol_g[order], ew_g[order]
    rstart = np.searchsorted(erow_s, np.arange(NIT + 1)).astype(np.int64)

    core_pos = []
    maxwin = 0
    for c in range(NC):
        it_flat = items[c * BLOC:(c + 1) * BLOC].reshape(-1).astype(np.int64)
        cnts = np.where(it_flat == 0, 0, rstart[it_flat + 1] - rstart[it_flat])
        wcnt = cnts.reshape(NWINA, WINA).sum(1)
        maxwin = max(maxwin, int(wcnt.max()))
        core_pos.append((it_flat, cnts, wcnt))
    T = max(1, int(math.ceil(maxwin / H)))
    NT = NWINA * T

    per_core = []
    for c in range(NC):
        it_flat, cnts, wcnt = core_pos[c]
        total = int(cnts.sum())
        starts_pos = rstart[it_flat]
        excl = np.cumsum(cnts) - cnts  # exclusive prefix
        src = np.repeat(starts_pos - excl, cnts) + np.arange(total)
        pos_rep = np.repeat(np.arange(RL), cnts)

        ec = np.zeros((NWINA, T * H), np.int32)
        er = np.full((NWINA, T * H), 300.0, np.float32)
        evw = np.zeros((NWINA, T * H), np.float32)
        wb = np.zeros(NWINA + 1, np.int64)
        np.cumsum(wcnt, out=wb[1:])
        for w in range(NWINA):
            s, e = wb[w], wb[w + 1]
            n = e - s
            ec[w, :n] = ecol_s[src[s:e]]
            er[w, :n] = (pos_rep[s:e] - w * WINA).astype(np.float32)
            evw[w, :n] = ew_s[src[s:e]]
        ec2 = ec.reshape(NT, H).T
        er2 = er.reshape(NT, H).T
        ev2 = evw.reshape(NT, H).T
        sw = ((er2[:, :, None] == np.arange(WINA, dtype=np.float32)) *
              ev2[:, :, None]).astype(np.float16)

        bsl = slice(c * BLOC, (c + 1) * BLOC)
        it_loc = items[bsl]                      # [8, 50]
        len_loc = lens[bsl]
        pos_idx = np.arange(L)[None, :]
        rev = len_loc[:, None] - 1 - pos_idx
        rev = np.where(it_loc == 0, 0, rev).astype(np.int32)
        pad = (it_loc == 0)

        h0x = np.ascontiguousarray(emb[it_loc.reshape(-1)].T)
        pox = np.ascontiguousarray(pos_emb[rev.reshape(-1)].T)

        pm_c = packm0.copy()
        attm = np.where(pad, -30000.0, 0.0).astype(np.float32).reshape(1, RL)
        pm_c[:, _OF_M["attm"]:_OF_M["attm"] + RL] = attm
        colmask = (~pad).astype(np.float32).reshape(1, RL)
        pm_c[:, _OF_M["colm"]:_OF_M["colm"] + RL] = colmask
        lastsel = np.zeros((BLOC, L), np.float32)
        lastsel[np.arange(BLOC), len_loc - 1] = 1.0
        pm_c[:, _OF_M["lastsel"]:_OF_M["lastsel"] + RL] = lastsel.reshape(
            1, RL)
        npadl = np.tile((L - len_loc).astype(np.float32), 4)  # [32]
        pm_c[:, _OF_M["npadl"]:_OF_M["npadl"] + 32] = npadl[None, :]

        adjbd = np.zeros((BLOC // 2, 2 * L, 2 * L), np.float32)
        for j in range(BLOC // 2):
            for i in range(2):
                adjbd[j, i * L:(i + 1) * L, i * L:(i + 1) * L] = (
                    adj[c * BLOC + 2 * j + i].T)

        m = dict(rep)
        m["packf"] = packf
        m["packm"] = pm_c
        m["h0x"] = h0x
        m["pox"] = pox
        m["adjbd"] = adjbd
        m["eemb"] = np.ascontiguousarray(emb16[ec2])
        m["swt"] = np.ascontiguousarray(sw)
        per_core.append(m)
    return per_core, T


def kernel(_trace=False, **inputs):
    in_maps, T = _prep(inputs)
    if T not in _NC_CACHE:
        _NC_CACHE[T] = build_nc(T)
    nc = _NC_CACHE[T]
    res = run_bass_kernel_spmd(nc, in_maps, core_ids=list(range(NC)),
                               trace=_trace)
    rows = []
    for c in range(NC):
        s1 = res.results[c]["scores1"].reshape(H, NCH, BLOC)
        s1 = s1.transpose(2, 1, 0).reshape(BLOC, NPAD)
        rows.append(s1 + res.results[c]["t23"])
    scores = np.concatenate(rows, axis=0)[:, :NIT - 1]
    if _trace:
        return scores, res
    return scores
